# revision 7
# baseline (speedup 1.0000x reference)
"""Trainium2 Bass kernel for BatchLabelPropagation.

Per episode b (of 16), e=128 samples, c=512 channels:
  sq_dist = ||x_i - x_j||^2 / sqrt(c)                (pairwise, diag exactly 0)
  standardize sq_dist by GLOBAL (all-episode) masked mean/var (ddof=1)
  W = exp(-sq_dist), diag zeroed
  S = W * colscale_j,  colscale_j = 1/(1e-4 + rowsum(W)_j)
  P = inv(I - 0.2 S);  P rows L1-normalized;  out = log(P @ onehot + 1e-6)

Strategy: 8 NeuronCores, 2 episodes per core, pure data parallel, NO
collectives (a tiny AllReduce has a ~20us latency floor on TRN2). Two
launches with a 16-float host-side combine between them:

  L1 (per core): xt (c-major) -> Gram G = X X^T via PE over 4 K-chunks;
     row norms r via a PE ones-reduction of x^2; two K=1 outer-product
     matmuls accumulate -(r_i+r_j)/2 onto G in PSUM, so one masked DVE
     multiply by (1-eye)*(-2/sqrt(c)) yields sq_dist with an exactly-zero
     diagonal. Local stats with a shifted one-pass sum (s = 2*sqrt(c), the
     analytic mean): A = sum(sq), Q = sum((sq-s)^2). Outputs sq and (A, Q).

  host: A, Q summed over cores (f64); var = (Q_off - D^2/cnt)/(cnt-1)
     with D = A - cnt*s, Q_off = Q - b*e*s^2; nis = -1/sqrt(var).

  L2 (per core): W = exp(nis*sq) (one fused op for both episodes), diag
     zeroed via mask; t = 0.2/(1e-4 + rowsum); inverse applied to
     B = [onehot | ones] by a Neumann series (||0.2 S|| ~= 0.17, 5 terms
     reach the f32 floor): v <- B + W@(t*v), with B re-added inside PSUM
     via an identity-matmul seed. out = log(v[:,:5]/v[:,5] + 1e-6).
"""
import numpy as np

import concourse.bass as bass
import concourse.bacc as bacc
import concourse.tile as tile
from concourse import mybir
from concourse import bass_utils

NCORES = 8
B_FULL = 16
EP = B_FULL // NCORES  # episodes per core
E = 128
C = 512
KCHUNKS = C // 128
NCLASSES = 5
NB = NCLASSES + 1

ALPHA = 0.2
EPS_OUT = 1e-6
EPS_DIAG = 1e-4
NEUMANN_ITERS = 5

SHIFT = float(2.0 * np.sqrt(np.float64(C)))  # analytic mean of sq_dist
CNT = float(B_FULL * E * (E - 1))
SQC = float(1.0 / np.sqrt(np.float64(C)))

F32 = mybir.dt.float32
AF = mybir.ActivationFunctionType
ALU = mybir.AluOpType

_CACHE = {}


def _new_bacc(ncores):
    return bacc.Bacc(
        "TRN2",
        target_bir_lowering=False,
        debug=False,
        enable_asserts=True,
        num_devices=ncores,
    )


def _build_l1(ncores=NCORES):
    nc = _new_bacc(ncores)
    xt_d = nc.dram_tensor("xt", [EP, C, E], F32, kind="ExternalInput").ap()
    sq_d = nc.dram_tensor("sqout", [EP, E, E], F32, kind="ExternalOutput").ap()
    st_d = nc.dram_tensor("stats", [1, 2], F32, kind="ExternalOutput").ap()

    maskscale_np = ((1.0 - np.eye(E)) * (-2.0 * SQC)).astype(np.float32)

    with tile.TileContext(nc) as tc:
        with (
            tc.tile_pool(name="sb", bufs=1) as sb,
            tc.tile_pool(name="ps", bufs=1, space="PSUM") as ps,
        ):
            maskscale = sb.tile([E, E], F32, tag="maskscale")
            nc.gpsimd.dma_start(
                out=maskscale, in_=nc.inline_tensor(maskscale_np, name="c_maskscale").ap()
            )
            ones_col = sb.tile([E, 1], F32, tag="ones_col")
            nc.vector.memset(ones_col, 1.0)
            ones_row = sb.tile([1, E], F32, tag="ones_row")
            nc.vector.memset(ones_row, 1.0)
            shift_col = sb.tile([E, 1], F32, tag="shift_col")
            nc.vector.memset(shift_col, -SHIFT)

            # x transposed, both episodes in one DMA: (p=c%128, ep, k=c//128, e)
            xt = sb.tile([E, EP, KCHUNKS, E], F32, tag="xt")
            nc.sync.dma_start(out=xt, in_=xt_d.rearrange("ep (k p) e -> p ep k e", p=E))
            xtsq = sb.tile([E, EP, KCHUNKS, E], F32, tag="xtsq")
            nc.vector.tensor_mul(xtsq, xt, xt)

            # Gram per episode
            g_ps = []
            for ep in range(EP):
                g = ps.tile([E, E], F32, tag=f"g{ep}")
                for k in range(KCHUNKS):
                    nc.tensor.matmul(
                        g, xt[:, ep, k, :], xt[:, ep, k, :], start=(k == 0), stop=False
                    )
                g_ps.append(g)

            # row norms for both episodes: (1, EP*E) via ones-reduction
            r_ps = ps.tile([1, EP, E], F32, tag="r")
            for k in range(KCHUNKS):
                nc.tensor.matmul(
                    r_ps, ones_col, xtsq[:, :, k, :], start=(k == 0), stop=(k == KCHUNKS - 1)
                )
            rhalf = sb.tile([1, EP, E], F32, tag="rhalf")
            nc.scalar.activation(rhalf, r_ps, AF.Copy, scale=-0.5)

            sq = sb.tile([E, EP, E], F32, tag="sq")
            for ep in range(EP):
                nc.tensor.matmul(g_ps[ep], rhalf[0:1, ep, :], ones_row, start=False, stop=False)
                nc.tensor.matmul(g_ps[ep], ones_row, rhalf[0:1, ep, :], start=False, stop=True)
                # sq = (G - (r_i+r_j)/2) * (1-eye)*(-2/sqrt(c))
                nc.vector.tensor_mul(sq[:, ep, :], g_ps[ep], maskscale)

            # local stats: A = sum(sq) rowwise, Q = sum((sq - s)^2) rowwise
            stats = sb.tile([E, 2], F32, tag="stats")
            nc.vector.tensor_reduce(
                stats[:, 0:1], sq, axis=mybir.AxisListType.XY, op=ALU.add
            )
            qscratch = sb.tile([E, EP, E], F32, tag="qscratch")
            nc.scalar.activation(
                qscratch, sq, AF.Square, bias=shift_col[:, 0:1], accum_out=stats[:, 1:2]
            )
            red_ps = ps.tile([1, 2], F32, tag="red")
            nc.tensor.matmul(red_ps, ones_col, stats, start=True, stop=True)
            red_sb = sb.tile([1, 2], F32, tag="red_sb")
            nc.vector.tensor_copy(red_sb, red_ps)

            nc.sync.dma_start(out=sq_d.rearrange("ep i j -> i ep j"), in_=sq)
            nc.gpsimd.dma_start(out=st_d, in_=red_sb)

    nc.compile()
    return nc


def _build_l2(ncores=NCORES):
    nc = _new_bacc(ncores)
    sq_d = nc.dram_tensor("sqin", [EP, E, E], F32, kind="ExternalInput").ap()
    bm_d = nc.dram_tensor("bmat", [EP, E, NB], F32, kind="ExternalInput").ap()
    nis_d = nc.dram_tensor("nis", [1, 1], F32, kind="ExternalInput").ap()
    out_d = nc.dram_tensor("out", [EP, E, NCLASSES], F32, kind="ExternalOutput").ap()

    mask01_np = np.broadcast_to(
        (1.0 - np.eye(E, dtype=np.float32))[:, None, :], (E, EP, E)
    ).copy()
    eye_np = np.eye(E, dtype=np.float32)

    with tile.TileContext(nc) as tc:
        with (
            tc.tile_pool(name="sb", bufs=1) as sb,
            tc.tile_pool(name="ps", bufs=2, space="PSUM") as ps,
        ):
            sq = sb.tile([E, EP, E], F32, tag="sq")
            nc.sync.dma_start(out=sq, in_=sq_d.rearrange("ep i j -> i ep j"))
            mask01 = sb.tile([E, EP, E], F32, tag="mask01")
            nc.gpsimd.dma_start(
                out=mask01, in_=nc.inline_tensor(mask01_np, name="c_mask01").ap()
            )
            eye = sb.tile([E, E], F32, tag="eye")
            nc.gpsimd.dma_start(out=eye, in_=nc.inline_tensor(eye_np, name="c_eye").ap())
            bm = sb.tile([E, EP, NB], F32, tag="bm")
            nc.gpsimd.dma_start(out=bm, in_=bm_d.rearrange("ep i j -> i ep j"))
            nis_col = sb.tile([E, 1], F32, tag="nis_col")
            nis_bcast = bass.AP(
                tensor=nis_d.tensor, offset=nis_d.offset, ap=[[0, E], [1, 1]]
            )
            nc.gpsimd.dma_start(out=nis_col, in_=nis_bcast)
            lnbias_col = sb.tile([E, 1], F32, tag="lnbias_col")
            nc.vector.memset(lnbias_col, EPS_OUT)

            # W for both episodes in one op; zero diag via mask
            w = sb.tile([E, EP, E], F32, tag="w")
            nc.scalar.activation(w, sq, AF.Exp, scale=nis_col[:, 0:1])
            wz = sb.tile([E, EP, E], F32, tag="wz")
            nc.vector.tensor_mul(wz, w, mask01)

            # t = alpha / (1e-4 + rowsum(W))
            dcol = sb.tile([E, EP], F32, tag="dcol")
            nc.vector.tensor_reduce(dcol, wz, axis=mybir.AxisListType.X, op=ALU.add)
            dn = sb.tile([E, EP], F32, tag="dn")
            nc.vector.tensor_scalar_add(dn, dcol, EPS_DIAG)
            trec = sb.tile([E, EP], F32, tag="trec")
            nc.vector.reciprocal(trec, dn)
            ts = sb.tile([E, EP], F32, tag="ts")
            nc.vector.tensor_scalar_mul(ts, trec, ALPHA)

            outv = sb.tile([E, EP, NCLASSES], F32, tag="outv")
            for ep in range(EP):
                bslice = bm[:, ep, :]
                u = sb.tile([E, NB], F32, tag=f"u{ep}")
                nc.vector.tensor_scalar_mul(u, bslice, ts[:, ep : ep + 1])
                v_ps = None
                for it in range(NEUMANN_ITERS):
                    v_ps = ps.tile([E, NB], F32, tag=f"v{ep}")
                    nc.tensor.matmul(v_ps, eye, bslice, start=True, stop=False)
                    nc.tensor.matmul(v_ps, wz[:, ep, :], u, start=False, stop=True)
                    if it < NEUMANN_ITERS - 1:
                        u = sb.tile([E, NB], F32, tag=f"u{ep}")
                        nc.vector.tensor_scalar_mul(u, v_ps, ts[:, ep : ep + 1])
                recip_l1 = sb.tile([E, 1], F32, tag=f"rl1{ep}")
                nc.vector.reciprocal(recip_l1, v_ps[:, NCLASSES : NCLASSES + 1])
                y = sb.tile([E, NCLASSES], F32, tag=f"y{ep}")
                nc.vector.tensor_scalar_mul(y, v_ps[:, 0:NCLASSES], recip_l1[:, 0:1])
                nc.scalar.activation(outv[:, ep, :], y, AF.Ln, bias=lnbias_col[:, 0:1])

            nc.sync.dma_start(out=out_d.rearrange("ep i j -> i ep j"), in_=outv)

    nc.compile()
    return nc


def _get(name, builder):
    if name not in _CACHE:
        _CACHE[name] = builder()
    return _CACHE[name]


def _prepare_l1_in_maps(x):
    x = np.ascontiguousarray(np.asarray(x, dtype=np.float32))
    xt = np.ascontiguousarray(x.transpose(0, 2, 1))  # (b, c, e)
    return [
        {"xt": np.ascontiguousarray(xt[c * EP : (c + 1) * EP])} for c in range(NCORES)
    ]


def _host_combine(stats_list):
    A = float(np.sum([s[0, 0] for s in stats_list], dtype=np.float64))
    Q = float(np.sum([s[0, 1] for s in stats_list], dtype=np.float64))
    q_off = Q - B_FULL * E * SHIFT * SHIFT
    d = A - CNT * SHIFT
    var = (q_off - d * d / CNT) / (CNT - 1.0)
    return np.float32(-1.0 / np.sqrt(var))


def _prepare_l2_in_maps(res1, labels, nis):
    labels = np.asarray(labels)
    bmat = np.zeros((B_FULL, E, NB), np.float32)
    bmat[..., NCLASSES] = 1.0
    for j in range(NCLASSES):
        bmat[..., j] = (labels == j).astype(np.float32)
    nis_arr = np.full((1, 1), nis, np.float32)
    maps = []
    for c in range(NCORES):
        maps.append(
            {
                "sqin": res1[c]["sqout"],
                "bmat": np.ascontiguousarray(bmat[c * EP : (c + 1) * EP]),
                "nis": nis_arr,
            }
        )
    return maps


def run(inputs):
    nc1 = _get("l1", _build_l1)
    nc2 = _get("l2", _build_l2)
    core_ids = list(range(NCORES))
    res1 = bass_utils.run_bass_kernel_spmd(
        nc1, _prepare_l1_in_maps(inputs["x"]), core_ids=core_ids
    ).results
    nis = _host_combine([r["stats"] for r in res1])
    res2 = bass_utils.run_bass_kernel_spmd(
        nc2, _prepare_l2_in_maps(res1, inputs["labels"], nis), core_ids=core_ids
    ).results
    out = np.concatenate([res2[c]["out"] for c in range(NCORES)], axis=0)
    return out.astype(np.float32)


def kernel(x, labels, nclasses):
    assert int(nclasses) == NCLASSES
    return run({"x": x, "labels": labels})


def timeline_estimate(trace_prefix=None):
    """Cost-model (TimelineSim) per-core estimates for both launches."""
    from concourse.timeline_sim import TimelineSim
    from trails.perfetto import LazyPerfetto

    for meth in ("enable_explicit_ordering", "reserve_process_order", "add_counter"):
        if not hasattr(LazyPerfetto, meth):
            setattr(LazyPerfetto, meth, lambda self, *a, **k: None)

    durs = []
    for name, builder in (("l1", _build_l1), ("l2", _build_l2)):
        nc = builder(ncores=1)
        trace = trace_prefix is not None
        tl = TimelineSim(nc, trace=trace)
        dur = tl.simulate()
        if trace and tl.perfetto is not None:
            tl.perfetto.save(f"{trace_prefix}_{name}.pftrace")
        durs.append(dur)
    return durs


if __name__ == "__main__":
    rng = np.random.default_rng(0)
    x = rng.standard_normal((B_FULL, E, C)).astype(np.float32)
    labels = rng.integers(0, NCLASSES + 1, size=(B_FULL, E))
    out = kernel(x, labels, NCLASSES)
    print("out", out.shape, out.dtype, out.min(), out.max())


# revision 13
# speedup vs baseline: 1.2495x; 1.2495x over previous
"""Trainium2 Bass kernel for BatchLabelPropagation.

Per episode b (of 16), e=128 samples, c=512 channels:
  sq_dist = ||x_i - x_j||^2 / sqrt(c)                (pairwise, diag exactly 0)
  standardize sq_dist by GLOBAL (all-episode) masked mean/var (ddof=1)
  W = exp(-sq_dist), diag zeroed
  S = W * colscale_j,  colscale_j = 1/(1e-4 + rowsum(W)_j)
  P = inv(I - 0.2 S);  P rows L1-normalized;  out = log(P @ onehot + 1e-6)

Strategy: 8 NeuronCores, 2 episodes per core, pure data parallel, NO
collectives (a tiny AllReduce has a ~20us latency floor on TRN2). Two
launches with a tiny host-side stats combine between them:

  L1 (per core): xt (c-major) -> Gram G = X X^T on PE over 4 K-chunks.
     r = diag(G) extracted by a single DVE tensor_mask_reduce (per-row
     window [p, p+1) + max-reduce), transposed to a row via a PE matmul
     against the identity, and -(r_i+r_j)/2 accumulated onto G in PSUM by
     two K=1 outer-product matmuls. One masked DVE multiply by
     (1-eye)*(-2/sqrt(c)) then yields sq_dist with an exactly-zero
     diagonal. Local shifted one-pass stats (s = 2*sqrt(c), the analytic
     mean): per-row A = sum(sq), Q = sum((sq-s)^2), packed with sq into a
     single (128, 260) output.

  host: A, Q summed (f64); var = (Q_off - D^2/cnt)/(cnt-1) with
     D = A - cnt*s, Q_off = Q - b*e*s^2; nis = -1/sqrt(var). 16 floats of
     glue - everything heavy stays on device.

  L2 (per core): W = exp(nis*sq) (one op for both episodes); diag zeroed +
     row-summed by a fused tensor_tensor_reduce (seed 1e-4); the inverse is
     applied to B = [onehot | ones] by a Neumann series (||0.2 S|| ~ 0.17,
     5 terms reach the f32 floor): v <- B + W@(t*v), with B re-added inside
     PSUM via an identity-matmul seed. out = log(v[:,:5]/v[:,5] + 1e-6).
"""
import numpy as np

import concourse.bass as bass
import concourse.bacc as bacc
import concourse.tile as tile
from concourse import mybir
from concourse import bass_utils

NCORES = 8
B_FULL = 16
EP = B_FULL // NCORES  # episodes per core
E = 128
C = 512
KCHUNKS = C // 128
NCLASSES = 5
NB = NCLASSES + 1
SQW = EP * E  # sq columns in the packed L1 output
STW = SQW + 4  # + [A0, A1, Q0, Q1] stat columns

ALPHA = 0.2
EPS_OUT = 1e-6
EPS_DIAG = 1e-4
NEUMANN_ITERS = 5
FMIN = float(np.finfo(np.float32).min)

SHIFT = float(2.0 * np.sqrt(np.float64(C)))  # analytic mean of sq_dist
CNT = float(B_FULL * E * (E - 1))
SQC = float(1.0 / np.sqrt(np.float64(C)))

F32 = mybir.dt.float32
AF = mybir.ActivationFunctionType
ALU = mybir.AluOpType
AX = mybir.AxisListType

_CACHE = {}


def _new_bacc(ncores):
    return bacc.Bacc(
        "TRN2",
        target_bir_lowering=False,
        debug=False,
        enable_asserts=True,
        num_devices=ncores,
    )


def _build_l1(ncores=NCORES):
    nc = _new_bacc(ncores)
    xt_d = nc.dram_tensor("xt", [EP, C, E], F32, kind="ExternalInput").ap()
    out_d = nc.dram_tensor("sqstat", [E, STW], F32, kind="ExternalOutput").ap()

    maskscale_np = ((1.0 - np.eye(E)) * (-2.0 * SQC)).astype(np.float32)
    eye_np = np.eye(E, dtype=np.float32)
    dlo_np = np.arange(E, dtype=np.float32).reshape(E, 1)
    dhi_np = dlo_np + 1.0

    with tile.TileContext(nc) as tc:
        with (
            tc.tile_pool(name="sb", bufs=1) as sb,
            tc.tile_pool(name="ps", bufs=1, space="PSUM") as ps,
        ):
            # small consts first (memsets are cheap and dependency-free)
            ones_row = sb.tile([1, E], F32, tag="ones_row")
            nc.vector.memset(ones_row, 1.0)
            shift_col = sb.tile([E, 1], F32, tag="shift_col")
            nc.vector.memset(shift_col, -SHIFT)
            # dependency-free dummy activations pull the ACT table load to t=0
            dummy = sb.tile([1, 1], F32, tag="dummy")
            nc.scalar.activation(dummy, shift_col[0:1, 0:1], AF.Square)

            maskscale = sb.tile([E, E], F32, tag="maskscale")
            nc.gpsimd.dma_start(
                out=maskscale, in_=nc.inline_tensor(maskscale_np, name="c_maskscale").ap()
            )
            eye = sb.tile([E, E], F32, tag="eye")
            nc.gpsimd.dma_start(out=eye, in_=nc.inline_tensor(eye_np, name="c_eye").ap())

            # x transposed, one DMA per episode so Gram can start early
            xt = []
            for ep in range(EP):
                t = sb.tile([E, KCHUNKS, E], F32, tag=f"xt{ep}")
                nc.sync.dma_start(out=t, in_=xt_d[ep].rearrange("(k p) e -> p k e", p=E))
                xt.append(t)

            out_sb = sb.tile([E, STW], F32, tag="out_sb")
            rpos = sb.tile([E, EP], F32, tag="rpos")
            rneg = sb.tile([E, EP], F32, tag="rneg")

            g_ps = []
            for ep in range(EP):
                g = ps.tile([E, E], F32, tag=f"g{ep}")
                for k in range(KCHUNKS):
                    nc.tensor.matmul(
                        g, xt[ep][:, k, :], xt[ep][:, k, :],
                        start=(k == 0), stop=(k == KCHUNKS - 1),
                    )
                g_ps.append(g)
                # r = diag(G) = rowsum(G * eye)
                dscratch = sb.tile([E, E], F32, tag="dscratch")
                nc.vector.tensor_mul(dscratch, g, eye)
                nc.vector.tensor_reduce(
                    rpos[:, ep : ep + 1], dscratch, axis=AX.X, op=ALU.add
                )

            nc.vector.tensor_scalar_mul(rneg, rpos, -0.5)

            for ep in range(EP):
                # transpose -r/2 to a row via eye-matmul, then broadcast it
                # down the partitions with a rank-1 outer product
                rr_ps = ps.tile([1, E], F32, tag=f"rr{ep}")
                nc.tensor.matmul(rr_ps, rneg[:, ep : ep + 1], eye, start=True, stop=True)
                rr = sb.tile([1, E], F32, tag=f"rrow{ep}")
                nc.vector.tensor_copy(rr, rr_ps)
                o_ps = ps.tile([E, E], F32, tag=f"o{ep}")
                nc.tensor.matmul(o_ps, ones_row, rr, start=True, stop=True)

                # sq = (G - r_i/2 - r_j/2) * (1-eye)*(-2/sqrt(c)); diag exactly 0
                # (only one PSUM input allowed per DVE op, so three steps)
                t1 = sb.tile([E, E], F32, tag="t1")
                nc.vector.tensor_scalar_add(t1, g_ps[ep], rneg[:, ep : ep + 1])
                t2 = sb.tile([E, E], F32, tag="t2")
                nc.vector.tensor_add(t2, t1, o_ps)
                sq_slice = out_sb[:, ep * E : (ep + 1) * E]
                nc.vector.tensor_mul(sq_slice, t2, maskscale)
                nc.vector.tensor_reduce(
                    out_sb[:, SQW + ep : SQW + ep + 1], sq_slice, axis=AX.X, op=ALU.add
                )
                qscratch = sb.tile([E, E], F32, tag="qscratch")
                nc.scalar.activation(
                    qscratch, sq_slice, AF.Square,
                    bias=shift_col[:, 0:1],
                    accum_out=out_sb[:, SQW + 2 + ep : SQW + 3 + ep],
                )

            nc.sync.dma_start(out=out_d, in_=out_sb)

    nc.compile()
    return nc


def _build_l2(ncores=NCORES):
    nc = _new_bacc(ncores)
    sq_d = nc.dram_tensor("sqstat", [E, STW], F32, kind="ExternalInput").ap()
    bm_d = nc.dram_tensor("bmat", [EP, E, NB], F32, kind="ExternalInput").ap()
    nis_d = nc.dram_tensor("nis", [1, 1], F32, kind="ExternalInput").ap()
    out_d = nc.dram_tensor("out", [EP, E, NCLASSES], F32, kind="ExternalOutput").ap()

    mask01_np = np.broadcast_to(
        (1.0 - np.eye(E, dtype=np.float32))[:, None, :], (E, EP, E)
    ).copy()
    eye_np = np.eye(E, dtype=np.float32)

    with tile.TileContext(nc) as tc:
        with (
            tc.tile_pool(name="sb", bufs=1) as sb,
            tc.tile_pool(name="ps", bufs=2, space="PSUM") as ps,
        ):
            lnbias_col = sb.tile([E, 1], F32, tag="lnbias_col")
            nc.vector.memset(lnbias_col, EPS_OUT)
            # dependency-free dummies pull both ACT table loads to t=0
            dummy = sb.tile([1, 1], F32, tag="dummy")
            nc.scalar.activation(dummy, lnbias_col[0:1, 0:1], AF.Exp)
            dummy2 = sb.tile([1, 1], F32, tag="dummy2")
            nc.scalar.activation(
                dummy2, lnbias_col[0:1, 0:1], AF.Ln, bias=lnbias_col[0:1, 0:1]
            )

            # nis first (tiny, gates exp), then consts by order of need
            nis_col = sb.tile([E, 1], F32, tag="nis_col")
            nis_bcast = bass.AP(
                tensor=nis_d.tensor, offset=nis_d.offset, ap=[[0, E], [1, 1]]
            )
            nc.gpsimd.dma_start(out=nis_col, in_=nis_bcast)
            mask01 = sb.tile([E, EP, E], F32, tag="mask01")
            nc.gpsimd.dma_start(
                out=mask01, in_=nc.inline_tensor(mask01_np, name="c_mask01").ap()
            )
            bm = sb.tile([E, EP, NB], F32, tag="bm")
            nc.gpsimd.dma_start(out=bm, in_=bm_d.rearrange("ep i j -> i ep j"))

            sq = sb.tile([E, EP, E], F32, tag="sq")
            nc.sync.dma_start(
                out=sq, in_=sq_d[:, 0:SQW].rearrange("p (ep e) -> p ep e", ep=EP)
            )
            eye = sb.tile([E, E], F32, tag="eye")
            nc.sync.dma_start(out=eye, in_=nc.inline_tensor(eye_np, name="c_eye").ap())

            # W for both episodes in one op
            w = sb.tile([E, EP, E], F32, tag="w")
            nc.scalar.activation(w, sq, AF.Exp, scale=nis_col[:, 0:1])
            # wz = W*(1-eye); d = 1e-4 + rowsum(wz)
            wz = sb.tile([E, EP, E], F32, tag="wz")
            nc.vector.tensor_mul(wz, w, mask01)
            dcol = sb.tile([E, EP], F32, tag="dcol")
            nc.vector.tensor_reduce(dcol, wz, axis=AX.X, op=ALU.add)
            dn = sb.tile([E, EP], F32, tag="dn")
            nc.vector.tensor_scalar_add(dn, dcol, EPS_DIAG)
            trec = sb.tile([E, EP], F32, tag="trec")
            nc.vector.reciprocal(trec, dn)
            ts = sb.tile([E, EP], F32, tag="ts")
            nc.vector.tensor_scalar_mul(ts, trec, ALPHA)

            y_both = sb.tile([E, EP, NCLASSES], F32, tag="y_both")
            for ep in range(EP):
                bslice = bm[:, ep, :]
                tslice = ts[:, ep : ep + 1]
                u = sb.tile([E, NB], F32, tag=f"u{ep}")
                nc.vector.tensor_scalar_mul(u, bslice, tslice)
                v_ps = None
                for it in range(NEUMANN_ITERS):
                    v_ps = ps.tile([E, NB], F32, tag=f"v{ep}")
                    nc.tensor.matmul(v_ps, eye, bslice, start=True, stop=False)
                    nc.tensor.matmul(v_ps, wz[:, ep, :], u, start=False, stop=True)
                    if it < NEUMANN_ITERS - 1:
                        u = sb.tile([E, NB], F32, tag=f"u{ep}")
                        nc.vector.tensor_scalar_mul(u, v_ps, tslice)
                recip_l1 = sb.tile([E, 1], F32, tag=f"rl1{ep}")
                nc.vector.reciprocal(recip_l1, v_ps[:, NCLASSES : NCLASSES + 1])
                nc.vector.tensor_scalar_mul(
                    y_both[:, ep, :], v_ps[:, 0:NCLASSES], recip_l1[:, 0:1]
                )
            outv = sb.tile([E, EP, NCLASSES], F32, tag="outv")
            nc.scalar.activation(outv, y_both, AF.Ln, bias=lnbias_col[:, 0:1])
            nc.sync.dma_start(out=out_d.rearrange("ep i j -> i ep j"), in_=outv)

    nc.compile()
    return nc


def _get(name, builder):
    if name not in _CACHE:
        _CACHE[name] = builder()
    return _CACHE[name]


def _prepare_l1_in_maps(x):
    x = np.ascontiguousarray(np.asarray(x, dtype=np.float32))
    xt = np.ascontiguousarray(x.transpose(0, 2, 1))  # (b, c, e)
    return [
        {"xt": np.ascontiguousarray(xt[c * EP : (c + 1) * EP])} for c in range(NCORES)
    ]


def _host_combine(sqstat_list):
    st = np.stack([s[:, SQW:] for s in sqstat_list]).astype(np.float64)  # (cores,E,4)
    A = float(st[..., 0:EP].sum())
    Q = float(st[..., EP : 2 * EP].sum())
    q_off = Q - B_FULL * E * SHIFT * SHIFT
    d = A - CNT * SHIFT
    var = (q_off - d * d / CNT) / (CNT - 1.0)
    return np.float32(-1.0 / np.sqrt(var))


def _prepare_l2_in_maps(res1, labels, nis):
    labels = np.asarray(labels)
    bmat = np.zeros((B_FULL, E, NB), np.float32)
    bmat[..., NCLASSES] = 1.0
    for j in range(NCLASSES):
        bmat[..., j] = (labels == j).astype(np.float32)
    nis_arr = np.full((1, 1), nis, np.float32)
    maps = []
    for c in range(NCORES):
        maps.append(
            {
                "sqstat": res1[c]["sqstat"],
                "bmat": np.ascontiguousarray(bmat[c * EP : (c + 1) * EP]),
                "nis": nis_arr,
            }
        )
    return maps


def run(inputs):
    nc1 = _get("l1", _build_l1)
    nc2 = _get("l2", _build_l2)
    core_ids = list(range(NCORES))
    res1 = bass_utils.run_bass_kernel_spmd(
        nc1, _prepare_l1_in_maps(inputs["x"]), core_ids=core_ids
    ).results
    nis = _host_combine([r["sqstat"] for r in res1])
    res2 = bass_utils.run_bass_kernel_spmd(
        nc2, _prepare_l2_in_maps(res1, inputs["labels"], nis), core_ids=core_ids
    ).results
    out = np.concatenate([res2[c]["out"] for c in range(NCORES)], axis=0)
    return out.astype(np.float32)


def kernel(x, labels, nclasses):
    assert int(nclasses) == NCLASSES
    return run({"x": x, "labels": labels})


def timeline_estimate(trace_prefix=None):
    """Cost-model (TimelineSim) per-core estimates for both launches."""
    from concourse.timeline_sim import TimelineSim
    from trails.perfetto import LazyPerfetto

    for meth in ("enable_explicit_ordering", "reserve_process_order", "add_counter"):
        if not hasattr(LazyPerfetto, meth):
            setattr(LazyPerfetto, meth, lambda self, *a, **k: None)

    durs = []
    for name, builder in (("l1", _build_l1), ("l2", _build_l2)):
        nc = builder(ncores=1)
        trace = trace_prefix is not None
        tl = TimelineSim(nc, trace=trace)
        dur = tl.simulate()
        if trace and tl.perfetto is not None:
            tl.perfetto.save(f"{trace_prefix}_{name}.pftrace")
        durs.append(dur)
    return durs


if __name__ == "__main__":
    rng = np.random.default_rng(0)
    x = rng.standard_normal((B_FULL, E, C)).astype(np.float32)
    labels = rng.integers(0, NCLASSES + 1, size=(B_FULL, E))
    out = kernel(x, labels, NCLASSES)
    print("out", out.shape, out.dtype, out.min(), out.max())


# revision 14
# speedup vs baseline: 1.4294x; 1.1440x over previous
"""Trainium2 Bass kernel for BatchLabelPropagation.

Per episode b (of 16), e=128 samples, c=512 channels:
  sq_dist = ||x_i - x_j||^2 / sqrt(c)                (pairwise, diag exactly 0)
  standardize sq_dist by GLOBAL (all-episode) masked mean/var (ddof=1)
  W = exp(-sq_dist), diag zeroed
  S = W * colscale_j,  colscale_j = 1/(1e-4 + rowsum(W)_j)
  P = inv(I - 0.2 S);  P rows L1-normalized;  out = log(P @ onehot + 1e-6)

Strategy: 8 NeuronCores, 2 episodes per core, pure data parallel, NO
collectives (a tiny AllReduce has a ~20us latency floor on TRN2). Two
launches with a tiny host-side stats combine between them:

  L1 (per core): xt (c-major) -> Gram G = X X^T on PE over 4 K-chunks.
     r = diag(G) = rowsum(G * (-eye/2)) on DVE; transposed to a row by a PE
     matmul against k*eye (k = -2/sqrt(c)) and broadcast down the
     partitions by a K=1 outer product with ones. sq = (G + rneg_i)*k + o
     needs no (1-eye) mask: the diagonal cancels EXACTLY in f32 because
     r_i is copied from G_ii (a - a/2 - a/2 == 0, and fl(-x*k) == -fl(x*k)).
     Local shifted one-pass stats (s = 2*sqrt(c), the analytic mean):
     per-row A = sum(sq) (DVE), Q = sum((sq-s)^2) (ACT Square+accum),
     packed with sq into a single (128, 260) output.

  host: A, Q summed (f64); var = (Q_off - D^2/cnt)/(cnt-1) with
     D = A - cnt*s, Q_off = Q - b*e*s^2; nis = -1/sqrt(var). 16 floats of
     glue - everything heavy stays on device.

  L2 (per core): W = exp(nis*sq) (one op for both episodes); diag zeroed
     via a (1-eye) mask then row-summed; the inverse is
     applied to B = [onehot | ones] by a Neumann series (||0.2 S|| ~ 0.17,
     5 terms reach the f32 floor): v <- B + W@(t*v), with B re-added inside
     PSUM via an identity-matmul seed. out = log(v[:,:5]/v[:,5] + 1e-6).
"""
import numpy as np

import concourse.bass as bass
import concourse.bacc as bacc
import concourse.tile as tile
from concourse import mybir
from concourse import bass_utils

NCORES = 8
B_FULL = 16
EP = B_FULL // NCORES  # episodes per core
E = 128
C = 512
KCHUNKS = C // 128
NCLASSES = 5
NB = NCLASSES + 1
SQW = EP * E  # sq columns in the packed L1 output
STW = SQW + 4  # + [A0, A1, Q0, Q1] stat columns

ALPHA = 0.2
EPS_OUT = 1e-6
EPS_DIAG = 1e-4
NEUMANN_ITERS = 5
FMIN = float(np.finfo(np.float32).min)

SHIFT = float(2.0 * np.sqrt(np.float64(C)))  # analytic mean of sq_dist
CNT = float(B_FULL * E * (E - 1))
SQC = float(1.0 / np.sqrt(np.float64(C)))

F32 = mybir.dt.float32
AF = mybir.ActivationFunctionType
ALU = mybir.AluOpType
AX = mybir.AxisListType

_CACHE = {}


def _new_bacc(ncores):
    return bacc.Bacc(
        "TRN2",
        target_bir_lowering=False,
        debug=False,
        enable_asserts=True,
        num_devices=ncores,
    )


def _build_l1(ncores=NCORES):
    nc = _new_bacc(ncores)
    xt_d = nc.dram_tensor("xt", [EP, C, E], F32, kind="ExternalInput").ap()
    out_d = nc.dram_tensor("sqstat", [E, STW], F32, kind="ExternalOutput").ap()

    eyescale_np = (np.eye(E) * -0.5).astype(np.float32)
    eyk_np = (np.eye(E) * (-2.0 * SQC)).astype(np.float32)

    with tile.TileContext(nc) as tc:
        with (
            tc.tile_pool(name="sb", bufs=1) as sb,
            tc.tile_pool(name="ps", bufs=1, space="PSUM") as ps,
        ):
            # small consts first (memsets are cheap and dependency-free)
            ones_row = sb.tile([1, E], F32, tag="ones_row")
            nc.vector.memset(ones_row, 1.0)
            shift_col = sb.tile([E, 1], F32, tag="shift_col")
            nc.vector.memset(shift_col, -SHIFT)
            # dependency-free dummy activations pull the ACT table load to t=0
            dummy = sb.tile([1, 1], F32, tag="dummy")
            nc.scalar.activation(dummy, shift_col[0:1, 0:1], AF.Square)

            eyescale = sb.tile([E, E], F32, tag="eyescale")
            nc.gpsimd.dma_start(
                out=eyescale, in_=nc.inline_tensor(eyescale_np, name="c_eyescale").ap()
            )
            eyk = sb.tile([E, E], F32, tag="eyk")
            nc.gpsimd.dma_start(out=eyk, in_=nc.inline_tensor(eyk_np, name="c_eyk").ap())

            # x transposed; ep0 split in two DMAs so its Gram starts earlier
            h = KCHUNKS // 2
            xt0a = sb.tile([E, h, E], F32, tag="xt0a")
            xt0b = sb.tile([E, h, E], F32, tag="xt0b")
            xt_r = xt_d[0].rearrange("(k p) e -> p k e", p=E)
            nc.sync.dma_start(out=xt0a, in_=xt_r[:, 0:h, :])
            nc.sync.dma_start(out=xt0b, in_=xt_r[:, h:KCHUNKS, :])
            xt1 = sb.tile([E, KCHUNKS, E], F32, tag="xt1")
            nc.sync.dma_start(out=xt1, in_=xt_d[1].rearrange("(k p) e -> p k e", p=E))

            def xt_chunk(ep, k):
                if ep == 1:
                    return xt1[:, k, :]
                return (xt0a if k < h else xt0b)[:, k % h, :]

            out_sb = sb.tile([E, STW], F32, tag="out_sb")
            rneg = sb.tile([E, EP], F32, tag="rneg")

            g_ps = []
            for ep in range(EP):
                g = ps.tile([E, E], F32, tag=f"g{ep}")
                for k in range(KCHUNKS):
                    ck = xt_chunk(ep, k)
                    nc.tensor.matmul(g, ck, ck, start=(k == 0), stop=(k == KCHUNKS - 1))
                g_ps.append(g)
                # rneg = -r/2 = rowsum(G * (-eye/2))
                dscratch = sb.tile([E, E], F32, tag="dscratch")
                nc.vector.tensor_mul(dscratch, g, eyescale)
                nc.vector.tensor_reduce(
                    rneg[:, ep : ep + 1], dscratch, axis=AX.X, op=ALU.add
                )

            for ep in range(EP):
                # row of rneg*k via the scaled-eye matmul, broadcast down the
                # partitions with a K=1 outer product against ones
                rr_ps = ps.tile([1, E], F32, tag=f"rr{ep}")
                nc.tensor.matmul(rr_ps, rneg[:, ep : ep + 1], eyk, start=True, stop=True)
                rr = sb.tile([1, E], F32, tag=f"rrow{ep}")
                nc.vector.tensor_copy(rr, rr_ps)
                o_ps = ps.tile([E, E], F32, tag=f"o{ep}")
                nc.tensor.matmul(o_ps, ones_row, rr, start=True, stop=True)

                # sq = (G + rneg_i)*k + o ; diagonal cancels exactly
                t1 = sb.tile([E, E], F32, tag="t1")
                nc.vector.tensor_scalar(
                    t1, g_ps[ep], rneg[:, ep : ep + 1], -2.0 * SQC,
                    op0=ALU.add, op1=ALU.mult,
                )
                sq_slice = out_sb[:, ep * E : (ep + 1) * E]
                nc.vector.tensor_add(sq_slice, t1, o_ps)
                nc.vector.tensor_reduce(
                    out_sb[:, SQW + ep : SQW + ep + 1], sq_slice, axis=AX.X, op=ALU.add
                )
                qscratch = sb.tile([E, E], F32, tag="qscratch")
                nc.scalar.activation(
                    qscratch, sq_slice, AF.Square,
                    bias=shift_col[:, 0:1],
                    accum_out=out_sb[:, SQW + 2 + ep : SQW + 3 + ep],
                )

            nc.sync.dma_start(out=out_d, in_=out_sb)

    nc.compile()
    return nc


def _build_l2(ncores=NCORES):
    nc = _new_bacc(ncores)
    sq_d = nc.dram_tensor("sqstat", [E, STW], F32, kind="ExternalInput").ap()
    bm_d = nc.dram_tensor("bmat", [EP, E, NB], F32, kind="ExternalInput").ap()
    nis_d = nc.dram_tensor("nis", [1, 1], F32, kind="ExternalInput").ap()
    out_d = nc.dram_tensor("out", [EP, E, NCLASSES], F32, kind="ExternalOutput").ap()

    mask01_np = np.broadcast_to(
        (1.0 - np.eye(E, dtype=np.float32))[:, None, :], (E, EP, E)
    ).copy()
    eye_np = np.eye(E, dtype=np.float32)

    with tile.TileContext(nc) as tc:
        with (
            tc.tile_pool(name="sb", bufs=1) as sb,
            tc.tile_pool(name="ps", bufs=2, space="PSUM") as ps,
        ):
            lnbias_col = sb.tile([E, 1], F32, tag="lnbias_col")
            nc.vector.memset(lnbias_col, EPS_OUT)
            # dependency-free dummies pull the ACT table loads forward; Ln
            # first so the last (resident) set is the one exp needs
            dummy = sb.tile([1, 1], F32, tag="dummy")
            nc.scalar.activation(
                dummy, lnbias_col[0:1, 0:1], AF.Ln, bias=lnbias_col[0:1, 0:1]
            )
            dummy2 = sb.tile([1, 1], F32, tag="dummy2")
            nc.scalar.activation(dummy2, lnbias_col[0:1, 0:1], AF.Exp)

            # nis on the HWDGE path right behind sq (tiny, gates exp)
            nis_col = sb.tile([E, 1], F32, tag="nis_col")
            nis_bcast = bass.AP(
                tensor=nis_d.tensor, offset=nis_d.offset, ap=[[0, E], [1, 1]]
            )
            nc.sync.dma_start(out=nis_col, in_=nis_bcast)
            mask01 = sb.tile([E, EP, E], F32, tag="mask01")
            nc.gpsimd.dma_start(
                out=mask01, in_=nc.inline_tensor(mask01_np, name="c_mask01").ap()
            )
            bm = sb.tile([E, EP, NB], F32, tag="bm")
            nc.gpsimd.dma_start(out=bm, in_=bm_d.rearrange("ep i j -> i ep j"))

            sq = sb.tile([E, EP, E], F32, tag="sq")
            nc.sync.dma_start(
                out=sq, in_=sq_d[:, 0:SQW].rearrange("p (ep e) -> p ep e", ep=EP)
            )
            eye = sb.tile([E, E], F32, tag="eye")
            nc.sync.dma_start(out=eye, in_=nc.inline_tensor(eye_np, name="c_eye").ap())

            # W for both episodes in one op
            w = sb.tile([E, EP, E], F32, tag="w")
            nc.scalar.activation(w, sq, AF.Exp, scale=nis_col[:, 0:1])
            # wz = W*(1-eye); d = 1e-4 + rowsum(wz)
            wz = sb.tile([E, EP, E], F32, tag="wz")
            nc.vector.tensor_mul(wz, w, mask01)
            dcol = sb.tile([E, EP], F32, tag="dcol")
            nc.vector.tensor_reduce(dcol, wz, axis=AX.X, op=ALU.add)
            dn = sb.tile([E, EP], F32, tag="dn")
            nc.vector.tensor_scalar_add(dn, dcol, EPS_DIAG)
            trec = sb.tile([E, EP], F32, tag="trec")
            nc.vector.reciprocal(trec, dn)
            ts = sb.tile([E, EP], F32, tag="ts")
            nc.vector.tensor_scalar_mul(ts, trec, ALPHA)

            y_both = sb.tile([E, EP, NCLASSES], F32, tag="y_both")
            for ep in range(EP):
                bslice = bm[:, ep, :]
                tslice = ts[:, ep : ep + 1]
                u = sb.tile([E, NB], F32, tag=f"u{ep}")
                nc.vector.tensor_scalar_mul(u, bslice, tslice)
                v_ps = None
                for it in range(NEUMANN_ITERS):
                    v_ps = ps.tile([E, NB], F32, tag=f"v{ep}")
                    nc.tensor.matmul(v_ps, eye, bslice, start=True, stop=False)
                    nc.tensor.matmul(v_ps, wz[:, ep, :], u, start=False, stop=True)
                    if it < NEUMANN_ITERS - 1:
                        u = sb.tile([E, NB], F32, tag=f"u{ep}")
                        nc.vector.tensor_scalar_mul(u, v_ps, tslice)
                recip_l1 = sb.tile([E, 1], F32, tag=f"rl1{ep}")
                nc.vector.reciprocal(recip_l1, v_ps[:, NCLASSES : NCLASSES + 1])
                nc.vector.tensor_scalar_mul(
                    y_both[:, ep, :], v_ps[:, 0:NCLASSES], recip_l1[:, 0:1]
                )
            outv = sb.tile([E, EP, NCLASSES], F32, tag="outv")
            nc.scalar.activation(outv, y_both, AF.Ln, bias=lnbias_col[:, 0:1])
            nc.sync.dma_start(out=out_d.rearrange("ep i j -> i ep j"), in_=outv)

    nc.compile()
    return nc


def _get(name, builder):
    if name not in _CACHE:
        _CACHE[name] = builder()
    return _CACHE[name]


def _prepare_l1_in_maps(x):
    x = np.ascontiguousarray(np.asarray(x, dtype=np.float32))
    xt = np.ascontiguousarray(x.transpose(0, 2, 1))  # (b, c, e)
    return [
        {"xt": np.ascontiguousarray(xt[c * EP : (c + 1) * EP])} for c in range(NCORES)
    ]


def _host_combine(sqstat_list):
    st = np.stack([s[:, SQW:] for s in sqstat_list]).astype(np.float64)  # (cores,E,4)
    A = float(st[..., 0:EP].sum())
    Q = float(st[..., EP : 2 * EP].sum())
    q_off = Q - B_FULL * E * SHIFT * SHIFT
    d = A - CNT * SHIFT
    var = (q_off - d * d / CNT) / (CNT - 1.0)
    return np.float32(-1.0 / np.sqrt(var))


def _prepare_l2_in_maps(res1, labels, nis):
    labels = np.asarray(labels)
    bmat = np.zeros((B_FULL, E, NB), np.float32)
    bmat[..., NCLASSES] = 1.0
    for j in range(NCLASSES):
        bmat[..., j] = (labels == j).astype(np.float32)
    nis_arr = np.full((1, 1), nis, np.float32)
    maps = []
    for c in range(NCORES):
        maps.append(
            {
                "sqstat": res1[c]["sqstat"],
                "bmat": np.ascontiguousarray(bmat[c * EP : (c + 1) * EP]),
                "nis": nis_arr,
            }
        )
    return maps


def run(inputs):
    nc1 = _get("l1", _build_l1)
    nc2 = _get("l2", _build_l2)
    core_ids = list(range(NCORES))
    res1 = bass_utils.run_bass_kernel_spmd(
        nc1, _prepare_l1_in_maps(inputs["x"]), core_ids=core_ids
    ).results
    nis = _host_combine([r["sqstat"] for r in res1])
    res2 = bass_utils.run_bass_kernel_spmd(
        nc2, _prepare_l2_in_maps(res1, inputs["labels"], nis), core_ids=core_ids
    ).results
    out = np.concatenate([res2[c]["out"] for c in range(NCORES)], axis=0)
    return out.astype(np.float32)


def kernel(x, labels, nclasses):
    assert int(nclasses) == NCLASSES
    return run({"x": x, "labels": labels})


def timeline_estimate(trace_prefix=None):
    """Cost-model (TimelineSim) per-core estimates for both launches."""
    from concourse.timeline_sim import TimelineSim
    from trails.perfetto import LazyPerfetto

    for meth in ("enable_explicit_ordering", "reserve_process_order", "add_counter"):
        if not hasattr(LazyPerfetto, meth):
            setattr(LazyPerfetto, meth, lambda self, *a, **k: None)

    durs = []
    for name, builder in (("l1", _build_l1), ("l2", _build_l2)):
        nc = builder(ncores=1)
        trace = trace_prefix is not None
        tl = TimelineSim(nc, trace=trace)
        dur = tl.simulate()
        if trace and tl.perfetto is not None:
            tl.perfetto.save(f"{trace_prefix}_{name}.pftrace")
        durs.append(dur)
    return durs


if __name__ == "__main__":
    rng = np.random.default_rng(0)
    x = rng.standard_normal((B_FULL, E, C)).astype(np.float32)
    labels = rng.integers(0, NCLASSES + 1, size=(B_FULL, E))
    out = kernel(x, labels, NCLASSES)
    print("out", out.shape, out.dtype, out.min(), out.max())


# revision 16
# speedup vs baseline: 1.4774x; 1.0336x over previous
"""Trainium2 Bass kernel for BatchLabelPropagation.

Per episode b (of 16), e=128 samples, c=512 channels:
  sq_dist = ||x_i - x_j||^2 / sqrt(c)                (pairwise, diag exactly 0)
  standardize sq_dist by GLOBAL (all-episode) masked mean/var (ddof=1)
  W = exp(-sq_dist), diag zeroed
  S = W * colscale_j,  colscale_j = 1/(1e-4 + rowsum(W)_j)
  P = inv(I - 0.2 S);  P rows L1-normalized;  out = log(P @ onehot + 1e-6)

Strategy: 8 NeuronCores, 2 episodes per core, pure data parallel, NO
collectives (a tiny AllReduce has a ~20us latency floor on TRN2). Two
launches with a tiny host-side stats combine between them:

  L1 (per core): xt (c-major) -> Gram G = X X^T on PE over 4 K-chunks.
     r = diag(G) = rowsum(G * (-eye/2)) on DVE; transposed to a row by a PE
     matmul against k*eye (k = -2/sqrt(c)) and broadcast down the
     partitions by a K=1 outer product with ones. sq = (G + rneg_i)*k + o
     needs no (1-eye) mask: the diagonal cancels EXACTLY in f32 because
     r_i is copied from G_ii (a - a/2 - a/2 == 0, and fl(-x*k) == -fl(x*k)).
     Local shifted one-pass stats (s = 2*sqrt(c), the analytic mean):
     per-row A = sum(sq) (DVE), Q = sum((sq-s)^2) (ACT Square+accum),
     packed with sq into a single (128, 260) output.

  host: A, Q summed (f64); var = (Q_off - D^2/cnt)/(cnt-1) with
     D = A - cnt*s, Q_off = Q - b*e*s^2; nis = -1/sqrt(var). 16 floats of
     glue - everything heavy stays on device.

  L2 (per core): W = exp(nis*sq) (one op for both episodes); diag zeroed
     via a (1-eye) mask then row-summed; the inverse is
     applied to B = [onehot | ones] by a Neumann series (||0.2 S|| ~ 0.17,
     4 terms reach the f32 floor): v <- B + W@(t*v), with B re-added inside
     PSUM via an identity-matmul seed. out = log(v[:,:5]/v[:,5] + 1e-6).
"""
import numpy as np

import concourse.bass as bass
import concourse.bacc as bacc
import concourse.tile as tile
from concourse import mybir
from concourse import bass_utils

NCORES = 8
B_FULL = 16
EP = B_FULL // NCORES  # episodes per core
E = 128
C = 512
KCHUNKS = C // 128
NCLASSES = 5
NB = NCLASSES + 1
SQW = EP * E  # sq columns in the packed L1 output
STW = SQW + 4  # + [A0, A1, Q0, Q1] stat columns

ALPHA = 0.2
EPS_OUT = 1e-6
EPS_DIAG = 1e-4
NEUMANN_ITERS = 4
FMIN = float(np.finfo(np.float32).min)

SHIFT = float(2.0 * np.sqrt(np.float64(C)))  # analytic mean of sq_dist
CNT = float(B_FULL * E * (E - 1))
SQC = float(1.0 / np.sqrt(np.float64(C)))

F32 = mybir.dt.float32
AF = mybir.ActivationFunctionType
ALU = mybir.AluOpType
AX = mybir.AxisListType

_CACHE = {}


def _new_bacc(ncores):
    return bacc.Bacc(
        "TRN2",
        target_bir_lowering=False,
        debug=False,
        enable_asserts=True,
        num_devices=ncores,
    )


def _build_l1(ncores=NCORES):
    nc = _new_bacc(ncores)
    xt_d = nc.dram_tensor("xt", [EP, C, E], F32, kind="ExternalInput").ap()
    out_d = nc.dram_tensor("sqstat", [E, STW], F32, kind="ExternalOutput").ap()

    eyescale_np = (np.eye(E) * -0.5).astype(np.float32)
    eyk_np = (np.eye(E) * (-2.0 * SQC)).astype(np.float32)

    with tile.TileContext(nc) as tc:
        with (
            tc.tile_pool(name="sb", bufs=1) as sb,
            tc.tile_pool(name="ps", bufs=1, space="PSUM") as ps,
        ):
            # small consts first (memsets are cheap and dependency-free)
            ones_row = sb.tile([1, E], F32, tag="ones_row")
            nc.vector.memset(ones_row, 1.0)
            shift_col = sb.tile([E, 1], F32, tag="shift_col")
            nc.vector.memset(shift_col, -SHIFT)
            # dependency-free dummy activations pull the ACT table load to t=0
            dummy = sb.tile([1, 1], F32, tag="dummy")
            nc.scalar.activation(dummy, shift_col[0:1, 0:1], AF.Square)

            eyescale = sb.tile([E, E], F32, tag="eyescale")
            nc.gpsimd.dma_start(
                out=eyescale, in_=nc.inline_tensor(eyescale_np, name="c_eyescale").ap()
            )
            eyk = sb.tile([E, E], F32, tag="eyk")
            nc.gpsimd.dma_start(out=eyk, in_=nc.inline_tensor(eyk_np, name="c_eyk").ap())

            # x transposed; ep0 split in two DMAs so its Gram starts earlier
            h = KCHUNKS // 2
            xt0a = sb.tile([E, h, E], F32, tag="xt0a")
            xt0b = sb.tile([E, h, E], F32, tag="xt0b")
            xt_r = xt_d[0].rearrange("(k p) e -> p k e", p=E)
            nc.sync.dma_start(out=xt0a, in_=xt_r[:, 0:h, :])
            nc.sync.dma_start(out=xt0b, in_=xt_r[:, h:KCHUNKS, :])
            xt1 = sb.tile([E, KCHUNKS, E], F32, tag="xt1")
            nc.sync.dma_start(out=xt1, in_=xt_d[1].rearrange("(k p) e -> p k e", p=E))

            def xt_chunk(ep, k):
                if ep == 1:
                    return xt1[:, k, :]
                return (xt0a if k < h else xt0b)[:, k % h, :]

            out_sb = sb.tile([E, STW], F32, tag="out_sb")
            rneg = sb.tile([E, EP], F32, tag="rneg")

            g_ps = []
            for ep in range(EP):
                g = ps.tile([E, E], F32, tag=f"g{ep}")
                for k in range(KCHUNKS):
                    ck = xt_chunk(ep, k)
                    nc.tensor.matmul(g, ck, ck, start=(k == 0), stop=(k == KCHUNKS - 1))
                g_ps.append(g)
                # rneg = -r/2 = rowsum(G * (-eye/2))
                dscratch = sb.tile([E, E], F32, tag="dscratch")
                nc.vector.tensor_mul(dscratch, g, eyescale)
                nc.vector.tensor_reduce(
                    rneg[:, ep : ep + 1], dscratch, axis=AX.X, op=ALU.add
                )

            for ep in range(EP):
                # row of rneg*k via the scaled-eye matmul, broadcast down the
                # partitions with a K=1 outer product against ones
                rr_ps = ps.tile([1, E], F32, tag=f"rr{ep}")
                nc.tensor.matmul(rr_ps, rneg[:, ep : ep + 1], eyk, start=True, stop=True)
                rr = sb.tile([1, E], F32, tag=f"rrow{ep}")
                nc.scalar.copy(rr, rr_ps)
                o_ps = ps.tile([E, E], F32, tag=f"o{ep}")
                nc.tensor.matmul(o_ps, ones_row, rr, start=True, stop=True)

                # sq = (G + rneg_i)*k + o ; diagonal cancels exactly
                t1 = sb.tile([E, E], F32, tag="t1")
                nc.vector.tensor_scalar(
                    t1, g_ps[ep], rneg[:, ep : ep + 1], -2.0 * SQC,
                    op0=ALU.add, op1=ALU.mult,
                )
                sq_slice = out_sb[:, ep * E : (ep + 1) * E]
                nc.vector.tensor_add(sq_slice, t1, o_ps)
                nc.vector.tensor_reduce(
                    out_sb[:, SQW + ep : SQW + ep + 1], sq_slice, axis=AX.X, op=ALU.add
                )
                qscratch = sb.tile([E, E], F32, tag="qscratch")
                nc.scalar.activation(
                    qscratch, sq_slice, AF.Square,
                    bias=shift_col[:, 0:1],
                    accum_out=out_sb[:, SQW + 2 + ep : SQW + 3 + ep],
                )

            nc.sync.dma_start(out=out_d, in_=out_sb)

    nc.compile()
    return nc


def _build_l2(ncores=NCORES):
    nc = _new_bacc(ncores)
    sq_d = nc.dram_tensor("sqstat", [E, STW], F32, kind="ExternalInput").ap()
    bm_d = nc.dram_tensor("bmat", [EP, E, NB], F32, kind="ExternalInput").ap()
    nis_d = nc.dram_tensor("nis", [1, 1], F32, kind="ExternalInput").ap()
    out_d = nc.dram_tensor("out", [EP, E, NCLASSES], F32, kind="ExternalOutput").ap()

    mask01_np = np.broadcast_to(
        (1.0 - np.eye(E, dtype=np.float32))[:, None, :], (E, EP, E)
    ).copy()
    eye_np = np.eye(E, dtype=np.float32)

    with tile.TileContext(nc) as tc:
        with (
            tc.tile_pool(name="sb", bufs=1) as sb,
            tc.tile_pool(name="ps", bufs=2, space="PSUM") as ps,
        ):
            lnbias_col = sb.tile([E, 1], F32, tag="lnbias_col")
            nc.vector.memset(lnbias_col, EPS_OUT)
            # dependency-free dummies pull the ACT table loads forward; Ln
            # first so the last (resident) set is the one exp needs
            dummy = sb.tile([1, 1], F32, tag="dummy")
            nc.scalar.activation(
                dummy, lnbias_col[0:1, 0:1], AF.Ln, bias=lnbias_col[0:1, 0:1]
            )
            dummy2 = sb.tile([1, 1], F32, tag="dummy2")
            nc.scalar.activation(dummy2, lnbias_col[0:1, 0:1], AF.Exp)

            # nis on the HWDGE path right behind sq (tiny, gates exp)
            nis_col = sb.tile([E, 1], F32, tag="nis_col")
            nis_bcast = bass.AP(
                tensor=nis_d.tensor, offset=nis_d.offset, ap=[[0, E], [1, 1]]
            )
            nc.sync.dma_start(out=nis_col, in_=nis_bcast)
            mask01 = sb.tile([E, EP, E], F32, tag="mask01")
            nc.gpsimd.dma_start(
                out=mask01, in_=nc.inline_tensor(mask01_np, name="c_mask01").ap()
            )
            bm = sb.tile([E, EP, NB], F32, tag="bm")
            nc.gpsimd.dma_start(out=bm, in_=bm_d.rearrange("ep i j -> i ep j"))

            sq = []
            for ep in range(EP):
                sq_t = sb.tile([E, E], F32, tag=f"sq{ep}")
                nc.sync.dma_start(out=sq_t, in_=sq_d[:, ep * E : (ep + 1) * E])
                sq.append(sq_t)
            eye = sb.tile([E, E], F32, tag="eye")
            nc.sync.dma_start(out=eye, in_=nc.inline_tensor(eye_np, name="c_eye").ap())

            wz = []
            ts = sb.tile([E, EP], F32, tag="ts")
            for ep in range(EP):
                w = sb.tile([E, E], F32, tag=f"w{ep}")
                nc.scalar.activation(w, sq[ep], AF.Exp, scale=nis_col[:, 0:1])
                wz_t = sb.tile([E, E], F32, tag=f"wz{ep}")
                nc.vector.tensor_mul(wz_t, w, mask01[:, ep, :])
                wz.append(wz_t)
                dcol = sb.tile([E, 1], F32, tag=f"dcol{ep}")
                nc.vector.tensor_reduce(dcol, wz_t, axis=AX.X, op=ALU.add)
                dn = sb.tile([E, 1], F32, tag=f"dn{ep}")
                nc.vector.tensor_scalar_add(dn, dcol, EPS_DIAG)
                trec = sb.tile([E, 1], F32, tag=f"trec{ep}")
                nc.vector.reciprocal(trec, dn)
                nc.vector.tensor_scalar_mul(ts[:, ep : ep + 1], trec, ALPHA)

            y_both = sb.tile([E, EP, NCLASSES], F32, tag="y_both")
            for ep in range(EP):
                bslice = bm[:, ep, :]
                tslice = ts[:, ep : ep + 1]
                u = sb.tile([E, NB], F32, tag=f"u{ep}")
                nc.vector.tensor_scalar_mul(u, bslice, tslice)
                v_ps = None
                for it in range(NEUMANN_ITERS):
                    v_ps = ps.tile([E, NB], F32, tag=f"v{ep}")
                    nc.tensor.matmul(v_ps, eye, bslice, start=True, stop=False)
                    nc.tensor.matmul(v_ps, wz[ep], u, start=False, stop=True)
                    if it < NEUMANN_ITERS - 1:
                        u = sb.tile([E, NB], F32, tag=f"u{ep}")
                        nc.vector.tensor_scalar_mul(u, v_ps, tslice)
                recip_l1 = sb.tile([E, 1], F32, tag=f"rl1{ep}")
                nc.vector.reciprocal(recip_l1, v_ps[:, NCLASSES : NCLASSES + 1])
                nc.vector.tensor_scalar_mul(
                    y_both[:, ep, :], v_ps[:, 0:NCLASSES], recip_l1[:, 0:1]
                )
            outv = sb.tile([E, EP, NCLASSES], F32, tag="outv")
            nc.scalar.activation(outv, y_both, AF.Ln, bias=lnbias_col[:, 0:1])
            nc.sync.dma_start(out=out_d.rearrange("ep i j -> i ep j"), in_=outv)

    nc.compile()
    return nc


def _get(name, builder):
    if name not in _CACHE:
        _CACHE[name] = builder()
    return _CACHE[name]


def _prepare_l1_in_maps(x):
    x = np.ascontiguousarray(np.asarray(x, dtype=np.float32))
    xt = np.ascontiguousarray(x.transpose(0, 2, 1))  # (b, c, e)
    return [
        {"xt": np.ascontiguousarray(xt[c * EP : (c + 1) * EP])} for c in range(NCORES)
    ]


def _host_combine(sqstat_list):
    st = np.stack([s[:, SQW:] for s in sqstat_list]).astype(np.float64)  # (cores,E,4)
    A = float(st[..., 0:EP].sum())
    Q = float(st[..., EP : 2 * EP].sum())
    q_off = Q - B_FULL * E * SHIFT * SHIFT
    d = A - CNT * SHIFT
    var = (q_off - d * d / CNT) / (CNT - 1.0)
    return np.float32(-1.0 / np.sqrt(var))


def _prepare_l2_in_maps(res1, labels, nis):
    labels = np.asarray(labels)
    bmat = np.zeros((B_FULL, E, NB), np.float32)
    bmat[..., NCLASSES] = 1.0
    for j in range(NCLASSES):
        bmat[..., j] = (labels == j).astype(np.float32)
    nis_arr = np.full((1, 1), nis, np.float32)
    maps = []
    for c in range(NCORES):
        maps.append(
            {
                "sqstat": res1[c]["sqstat"],
                "bmat": np.ascontiguousarray(bmat[c * EP : (c + 1) * EP]),
                "nis": nis_arr,
            }
        )
    return maps


def run(inputs):
    nc1 = _get("l1", _build_l1)
    nc2 = _get("l2", _build_l2)
    core_ids = list(range(NCORES))
    res1 = bass_utils.run_bass_kernel_spmd(
        nc1, _prepare_l1_in_maps(inputs["x"]), core_ids=core_ids
    ).results
    nis = _host_combine([r["sqstat"] for r in res1])
    res2 = bass_utils.run_bass_kernel_spmd(
        nc2, _prepare_l2_in_maps(res1, inputs["labels"], nis), core_ids=core_ids
    ).results
    out = np.concatenate([res2[c]["out"] for c in range(NCORES)], axis=0)
    return out.astype(np.float32)


def kernel(x, labels, nclasses):
    assert int(nclasses) == NCLASSES
    return run({"x": x, "labels": labels})


def timeline_estimate(trace_prefix=None):
    """Cost-model (TimelineSim) per-core estimates for both launches."""
    from concourse.timeline_sim import TimelineSim
    from trails.perfetto import LazyPerfetto

    for meth in ("enable_explicit_ordering", "reserve_process_order", "add_counter"):
        if not hasattr(LazyPerfetto, meth):
            setattr(LazyPerfetto, meth, lambda self, *a, **k: None)

    durs = []
    for name, builder in (("l1", _build_l1), ("l2", _build_l2)):
        nc = builder(ncores=1)
        trace = trace_prefix is not None
        tl = TimelineSim(nc, trace=trace)
        dur = tl.simulate()
        if trace and tl.perfetto is not None:
            tl.perfetto.save(f"{trace_prefix}_{name}.pftrace")
        durs.append(dur)
    return durs


if __name__ == "__main__":
    rng = np.random.default_rng(0)
    x = rng.standard_normal((B_FULL, E, C)).astype(np.float32)
    labels = rng.integers(0, NCLASSES + 1, size=(B_FULL, E))
    out = kernel(x, labels, NCLASSES)
    print("out", out.shape, out.dtype, out.min(), out.max())


# revision 18
# speedup vs baseline: 1.4905x; 1.0089x over previous
"""Trainium2 Bass kernel for BatchLabelPropagation.

Per episode b (of 16), e=128 samples, c=512 channels:
  sq_dist = ||x_i - x_j||^2 / sqrt(c)                (pairwise, diag exactly 0)
  standardize sq_dist by GLOBAL (all-episode) masked mean/var (ddof=1)
  W = exp(-sq_dist), diag zeroed
  S = W * colscale_j,  colscale_j = 1/(1e-4 + rowsum(W)_j)
  P = inv(I - 0.2 S);  P rows L1-normalized;  out = log(P @ onehot + 1e-6)

Strategy: 8 NeuronCores, 2 episodes per core, pure data parallel, NO
collectives (a tiny AllReduce has a ~20us latency floor on TRN2). Two
launches with a tiny host-side stats combine between them:

  L1 (per core): xt (c-major) -> Gram G = X X^T on PE over 4 K-chunks.
     r = diag(G) = rowsum(G * (-eye/2)) on DVE; transposed to a row by a PE
     matmul against k*eye (k = -2/sqrt(c)) and broadcast down the
     partitions by a K=1 outer product with ones. sq = (G + rneg_i)*k + o
     needs no (1-eye) mask: the diagonal cancels EXACTLY in f32 because
     r_i is copied from G_ii (a - a/2 - a/2 == 0, and fl(-x*k) == -fl(x*k)).
     Local shifted one-pass stats (s = 2*sqrt(c), the analytic mean):
     per-row A = sum(sq) (DVE), Q = sum((sq-s)^2) (ACT Square+accum),
     packed with sq into a single (128, 260) output.

  host: A, Q summed (f64); var = (Q_off - D^2/cnt)/(cnt-1) with
     D = A - cnt*s, Q_off = Q - b*e*s^2; nis = -1/sqrt(var). 16 floats of
     glue - everything heavy stays on device.

  L2 (per core): W = exp(nis*sq) (one op for both episodes); diag zeroed
     via a (1-eye) mask then row-summed; the inverse is
     applied to B = [onehot | ones] by a Neumann series (||0.2 S|| ~ 0.17,
     4 terms reach the f32 floor): v <- B + W@(t*v), with B re-added inside
     PSUM via an identity-matmul seed. out = log(v[:,:5]/v[:,5] + 1e-6).
"""
import numpy as np

import concourse.bass as bass
import concourse.bacc as bacc
import concourse.tile as tile
from concourse import mybir
from concourse import bass_utils

NCORES = 8
B_FULL = 16
EP = B_FULL // NCORES  # episodes per core
E = 128
C = 512
KCHUNKS = C // 128
NCLASSES = 5
NB = NCLASSES + 1
SQW = EP * E  # sq columns in the packed L1 output
STW = SQW + 4  # + [A0, A1, Q0, Q1] stat columns

ALPHA = 0.2
EPS_OUT = 1e-6
EPS_DIAG = 1e-4
NEUMANN_ITERS = 4

SHIFT = float(2.0 * np.sqrt(np.float64(C)))  # analytic mean of sq_dist
CNT = float(B_FULL * E * (E - 1))
SQC = float(1.0 / np.sqrt(np.float64(C)))

F32 = mybir.dt.float32
AF = mybir.ActivationFunctionType
ALU = mybir.AluOpType
AX = mybir.AxisListType

_CACHE = {}


def _new_bacc(ncores):
    return bacc.Bacc(
        "TRN2",
        target_bir_lowering=False,
        debug=False,
        enable_asserts=True,
        num_devices=ncores,
    )


def _build_l1(ncores=NCORES):
    nc = _new_bacc(ncores)
    xt_d = nc.dram_tensor("xt", [EP, C, E], F32, kind="ExternalInput").ap()
    out_d = nc.dram_tensor("sqstat", [E, STW], F32, kind="ExternalOutput").ap()

    eyescale_np = (np.eye(E) * -0.5).astype(np.float32)
    eyk_np = (np.eye(E) * (-2.0 * SQC)).astype(np.float32)

    with tile.TileContext(nc) as tc:
        with (
            tc.tile_pool(name="sb", bufs=1) as sb,
            tc.tile_pool(name="scr", bufs=2) as scr,
            tc.tile_pool(name="ps", bufs=1, space="PSUM") as ps,
        ):
            # small consts first (memsets are cheap and dependency-free)
            ones_row = sb.tile([1, E], F32, tag="ones_row")
            nc.vector.memset(ones_row, 1.0)
            shift_col = sb.tile([E, 1], F32, tag="shift_col")
            nc.vector.memset(shift_col, -SHIFT)
            # dependency-free dummy activations pull the ACT table load to t=0
            dummy = sb.tile([1, 1], F32, tag="dummy")
            nc.scalar.activation(dummy, shift_col[0:1, 0:1], AF.Square)

            eyescale = sb.tile([E, E], F32, tag="eyescale")
            nc.gpsimd.dma_start(
                out=eyescale, in_=nc.inline_tensor(eyescale_np, name="c_eyescale").ap()
            )
            eyk = sb.tile([E, E], F32, tag="eyk")
            nc.gpsimd.dma_start(out=eyk, in_=nc.inline_tensor(eyk_np, name="c_eyk").ap())

            # x transposed; ep0 split in two DMAs so its Gram starts earlier
            h = KCHUNKS // 2
            xt0a = sb.tile([E, h, E], F32, tag="xt0a")
            xt0b = sb.tile([E, h, E], F32, tag="xt0b")
            xt_r = xt_d[0].rearrange("(k p) e -> p k e", p=E)
            nc.sync.dma_start(out=xt0a, in_=xt_r[:, 0:h, :])
            nc.sync.dma_start(out=xt0b, in_=xt_r[:, h:KCHUNKS, :])
            xt1 = sb.tile([E, KCHUNKS, E], F32, tag="xt1")
            nc.sync.dma_start(out=xt1, in_=xt_d[1].rearrange("(k p) e -> p k e", p=E))

            def xt_chunk(ep, k):
                if ep == 1:
                    return xt1[:, k, :]
                return (xt0a if k < h else xt0b)[:, k % h, :]

            out_sb = sb.tile([E, STW], F32, tag="out_sb")
            rneg = sb.tile([E, EP], F32, tag="rneg")

            g_ps = []
            for ep in range(EP):
                g = ps.tile([E, E], F32, tag=f"g{ep}")
                for k in range(KCHUNKS):
                    ck = xt_chunk(ep, k)
                    nc.tensor.matmul(g, ck, ck, start=(k == 0), stop=(k == KCHUNKS - 1))
                g_ps.append(g)
                # rneg = -r/2 = rowsum(G * (-eye/2))
                dscratch = scr.tile([E, E], F32, tag="dscratch")
                nc.vector.tensor_mul(dscratch, g, eyescale)
                nc.vector.tensor_reduce(
                    rneg[:, ep : ep + 1], dscratch, axis=AX.X, op=ALU.add
                )

            for ep in range(EP):
                # row of rneg*k via the scaled-eye matmul, broadcast down the
                # partitions with a K=1 outer product against ones
                rr_ps = ps.tile([1, E], F32, tag=f"rr{ep}")
                nc.tensor.matmul(rr_ps, rneg[:, ep : ep + 1], eyk, start=True, stop=True)
                rr = sb.tile([1, E], F32, tag=f"rrow{ep}")
                nc.scalar.copy(rr, rr_ps)
                o_ps = ps.tile([E, E], F32, tag=f"o{ep}")
                nc.tensor.matmul(o_ps, ones_row, rr, start=True, stop=True)

                # sq = (G + rneg_i)*k + o ; diagonal cancels exactly
                t1 = scr.tile([E, E], F32, tag="t1")
                nc.vector.tensor_scalar(
                    t1, g_ps[ep], rneg[:, ep : ep + 1], -2.0 * SQC,
                    op0=ALU.add, op1=ALU.mult,
                )
                sq_slice = out_sb[:, ep * E : (ep + 1) * E]
                nc.vector.tensor_add(sq_slice, t1, o_ps)
                nc.vector.tensor_reduce(
                    out_sb[:, SQW + ep : SQW + ep + 1], sq_slice, axis=AX.X, op=ALU.add
                )
                qscratch = scr.tile([E, E], F32, tag="qscratch")
                nc.scalar.activation(
                    qscratch, sq_slice, AF.Square,
                    bias=shift_col[:, 0:1],
                    accum_out=out_sb[:, SQW + 2 + ep : SQW + 3 + ep],
                )

            nc.sync.dma_start(out=out_d, in_=out_sb)

    nc.compile()
    return nc


def _build_l2(ncores=NCORES):
    nc = _new_bacc(ncores)
    sq_d = nc.dram_tensor("sqstat", [E, STW], F32, kind="ExternalInput").ap()
    bm_d = nc.dram_tensor("bmat", [EP, E, NB], F32, kind="ExternalInput").ap()
    nis_d = nc.dram_tensor("nis", [1, 1], F32, kind="ExternalInput").ap()
    out_d = nc.dram_tensor("out", [EP, E, NCLASSES], F32, kind="ExternalOutput").ap()

    mask01_np = np.broadcast_to(
        (1.0 - np.eye(E, dtype=np.float32))[:, None, :], (E, EP, E)
    ).copy()
    eye_np = np.eye(E, dtype=np.float32)

    with tile.TileContext(nc) as tc:
        with (
            tc.tile_pool(name="sb", bufs=1) as sb,
            tc.tile_pool(name="ps", bufs=2, space="PSUM") as ps,
        ):
            lnbias_col = sb.tile([E, 1], F32, tag="lnbias_col")
            nc.vector.memset(lnbias_col, EPS_OUT)
            # dependency-free dummies pull the ACT table loads forward; Ln
            # first so the last (resident) set is the one exp needs
            dummy = sb.tile([1, 1], F32, tag="dummy")
            nc.scalar.activation(
                dummy, lnbias_col[0:1, 0:1], AF.Ln, bias=lnbias_col[0:1, 0:1]
            )
            dummy2 = sb.tile([1, 1], F32, tag="dummy2")
            nc.scalar.activation(dummy2, lnbias_col[0:1, 0:1], AF.Exp)

            # nis on the HWDGE path right behind sq (tiny, gates exp)
            nis_col = sb.tile([E, 1], F32, tag="nis_col")
            nis_bcast = bass.AP(
                tensor=nis_d.tensor, offset=nis_d.offset, ap=[[0, E], [1, 1]]
            )
            nc.sync.dma_start(out=nis_col, in_=nis_bcast)
            mask01 = sb.tile([E, EP, E], F32, tag="mask01")
            nc.gpsimd.dma_start(
                out=mask01, in_=nc.inline_tensor(mask01_np, name="c_mask01").ap()
            )
            bm = sb.tile([E, EP, NB], F32, tag="bm")
            nc.gpsimd.dma_start(out=bm, in_=bm_d.rearrange("ep i j -> i ep j"))

            sq = []
            for ep in range(EP):
                sq_t = sb.tile([E, E], F32, tag=f"sq{ep}")
                nc.sync.dma_start(out=sq_t, in_=sq_d[:, ep * E : (ep + 1) * E])
                sq.append(sq_t)
            eye = sb.tile([E, E], F32, tag="eye")
            nc.sync.dma_start(out=eye, in_=nc.inline_tensor(eye_np, name="c_eye").ap())

            wz = []
            ts = sb.tile([E, EP], F32, tag="ts")
            for ep in range(EP):
                w = sb.tile([E, E], F32, tag=f"w{ep}")
                nc.scalar.activation(w, sq[ep], AF.Exp, scale=nis_col[:, 0:1])
                wz_t = sb.tile([E, E], F32, tag=f"wz{ep}")
                nc.vector.tensor_mul(wz_t, w, mask01[:, ep, :])
                wz.append(wz_t)
                dcol = sb.tile([E, 1], F32, tag=f"dcol{ep}")
                nc.vector.tensor_reduce(dcol, wz_t, axis=AX.X, op=ALU.add)
                dn = sb.tile([E, 1], F32, tag=f"dn{ep}")
                nc.vector.tensor_scalar_add(dn, dcol, EPS_DIAG)
                trec = sb.tile([E, 1], F32, tag=f"trec{ep}")
                nc.vector.reciprocal(trec, dn)
                nc.vector.tensor_scalar_mul(ts[:, ep : ep + 1], trec, ALPHA)

            y_both = sb.tile([E, EP, NCLASSES], F32, tag="y_both")
            for ep in range(EP):
                bslice = bm[:, ep, :]
                tslice = ts[:, ep : ep + 1]
                u = sb.tile([E, NB], F32, tag=f"u{ep}")
                nc.vector.tensor_scalar_mul(u, bslice, tslice)
                v_ps = None
                for it in range(NEUMANN_ITERS):
                    v_ps = ps.tile([E, NB], F32, tag=f"v{ep}")
                    nc.tensor.matmul(v_ps, eye, bslice, start=True, stop=False)
                    nc.tensor.matmul(v_ps, wz[ep], u, start=False, stop=True)
                    if it < NEUMANN_ITERS - 1:
                        u = sb.tile([E, NB], F32, tag=f"u{ep}")
                        nc.vector.tensor_scalar_mul(u, v_ps, tslice)
                recip_l1 = sb.tile([E, 1], F32, tag=f"rl1{ep}")
                nc.vector.reciprocal(recip_l1, v_ps[:, NCLASSES : NCLASSES + 1])
                nc.vector.tensor_scalar_mul(
                    y_both[:, ep, :], v_ps[:, 0:NCLASSES], recip_l1[:, 0:1]
                )
            outv = sb.tile([E, EP, NCLASSES], F32, tag="outv")
            nc.scalar.activation(outv, y_both, AF.Ln, bias=lnbias_col[:, 0:1])
            nc.sync.dma_start(out=out_d.rearrange("ep i j -> i ep j"), in_=outv)

    nc.compile()
    return nc


def _get(name, builder):
    if name not in _CACHE:
        _CACHE[name] = builder()
    return _CACHE[name]


def _prepare_l1_in_maps(x):
    x = np.ascontiguousarray(np.asarray(x, dtype=np.float32))
    xt = np.ascontiguousarray(x.transpose(0, 2, 1))  # (b, c, e)
    return [
        {"xt": np.ascontiguousarray(xt[c * EP : (c + 1) * EP])} for c in range(NCORES)
    ]


def _host_combine(sqstat_list):
    st = np.stack([s[:, SQW:] for s in sqstat_list]).astype(np.float64)  # (cores,E,4)
    A = float(st[..., 0:EP].sum())
    Q = float(st[..., EP : 2 * EP].sum())
    q_off = Q - B_FULL * E * SHIFT * SHIFT
    d = A - CNT * SHIFT
    var = (q_off - d * d / CNT) / (CNT - 1.0)
    return np.float32(-1.0 / np.sqrt(var))


def _prepare_l2_in_maps(res1, labels, nis):
    labels = np.asarray(labels)
    bmat = np.zeros((B_FULL, E, NB), np.float32)
    bmat[..., NCLASSES] = 1.0
    for j in range(NCLASSES):
        bmat[..., j] = (labels == j).astype(np.float32)
    nis_arr = np.full((1, 1), nis, np.float32)
    maps = []
    for c in range(NCORES):
        maps.append(
            {
                "sqstat": res1[c]["sqstat"],
                "bmat": np.ascontiguousarray(bmat[c * EP : (c + 1) * EP]),
                "nis": nis_arr,
            }
        )
    return maps


def run(inputs):
    nc1 = _get("l1", _build_l1)
    nc2 = _get("l2", _build_l2)
    core_ids = list(range(NCORES))
    res1 = bass_utils.run_bass_kernel_spmd(
        nc1, _prepare_l1_in_maps(inputs["x"]), core_ids=core_ids
    ).results
    nis = _host_combine([r["sqstat"] for r in res1])
    res2 = bass_utils.run_bass_kernel_spmd(
        nc2, _prepare_l2_in_maps(res1, inputs["labels"], nis), core_ids=core_ids
    ).results
    out = np.concatenate([res2[c]["out"] for c in range(NCORES)], axis=0)
    return out.astype(np.float32)


def kernel(x, labels, nclasses):
    assert int(nclasses) == NCLASSES
    return run({"x": x, "labels": labels})


def timeline_estimate(trace_prefix=None):
    """Cost-model (TimelineSim) per-core estimates for both launches."""
    from concourse.timeline_sim import TimelineSim
    from trails.perfetto import LazyPerfetto

    for meth in ("enable_explicit_ordering", "reserve_process_order", "add_counter"):
        if not hasattr(LazyPerfetto, meth):
            setattr(LazyPerfetto, meth, lambda self, *a, **k: None)

    durs = []
    for name, builder in (("l1", _build_l1), ("l2", _build_l2)):
        nc = builder(ncores=1)
        trace = trace_prefix is not None
        tl = TimelineSim(nc, trace=trace)
        dur = tl.simulate()
        if trace and tl.perfetto is not None:
            tl.perfetto.save(f"{trace_prefix}_{name}.pftrace")
        durs.append(dur)
    return durs


if __name__ == "__main__":
    rng = np.random.default_rng(0)
    x = rng.standard_normal((B_FULL, E, C)).astype(np.float32)
    labels = rng.integers(0, NCLASSES + 1, size=(B_FULL, E))
    out = kernel(x, labels, NCLASSES)
    print("out", out.shape, out.dtype, out.min(), out.max())


# revision 20
# speedup vs baseline: 1.4929x; 1.0016x over previous
"""Trainium2 Bass kernel for BatchLabelPropagation.

Per episode b (of 16), e=128 samples, c=512 channels:
  sq_dist = ||x_i - x_j||^2 / sqrt(c)                (pairwise, diag exactly 0)
  standardize sq_dist by GLOBAL (all-episode) masked mean/var (ddof=1)
  W = exp(-sq_dist), diag zeroed
  S = W * colscale_j,  colscale_j = 1/(1e-4 + rowsum(W)_j)
  P = inv(I - 0.2 S);  P rows L1-normalized;  out = log(P @ onehot + 1e-6)

Strategy: 8 NeuronCores, 2 episodes per core, pure data parallel, NO
collectives (a tiny AllReduce has a ~20us latency floor on TRN2). Two
launches with a tiny host-side stats combine between them:

  L1 (per core): xt (c-major) -> Gram G = X X^T on PE over 4 K-chunks.
     r = diag(G) = rowsum(G * (-eye/2)) on DVE; transposed to a row by a PE
     matmul against k*eye (k = -2/sqrt(c)) and broadcast down the
     partitions by a K=1 outer product with ones. sq = (G + rneg_i)*k + o
     needs no (1-eye) mask: the diagonal cancels EXACTLY in f32 because
     r_i is copied from G_ii (a - a/2 - a/2 == 0, and fl(-x*k) == -fl(x*k)).
     Local shifted one-pass stats (s = 2*sqrt(c), the analytic mean):
     per-row A = sum(sq) (DVE), Q = sum((sq-s)^2) (ACT Square+accum),
     packed with sq into a single (128, 260) output.

  host: A, Q summed (f64); var = (Q_off - D^2/cnt)/(cnt-1) with
     D = A - cnt*s, Q_off = Q - b*e*s^2; nis = -1/sqrt(var). 16 floats of
     glue - everything heavy stays on device.

  L2 (per core): W = exp(nis*sq) (one op for both episodes); diag zeroed
     via a (1-eye) mask then row-summed; the inverse is
     applied to B = [onehot | ones] by a Neumann series (||0.2 S|| ~ 0.17,
     4 terms reach the f32 floor): v <- B + W@(t*v), with B re-added inside
     PSUM via an identity-matmul seed. out = log(v[:,:5]/v[:,5] + 1e-6).
"""
import numpy as np

import concourse.bass as bass
import concourse.bacc as bacc
import concourse.tile as tile
from concourse import mybir
from concourse import bass_utils

NCORES = 8
B_FULL = 16
EP = B_FULL // NCORES  # episodes per core
E = 128
C = 512
KCHUNKS = C // 128
NCLASSES = 5
NB = NCLASSES + 1
SQW = EP * E  # sq columns in the packed L1 output
STW = SQW + 4  # + [A0, A1, Q0, Q1] stat columns

ALPHA = 0.2
EPS_OUT = 1e-6
EPS_DIAG = 1e-4
NEUMANN_ITERS = 4

SHIFT = float(2.0 * np.sqrt(np.float64(C)))  # analytic mean of sq_dist
CNT = float(B_FULL * E * (E - 1))
SQC = float(1.0 / np.sqrt(np.float64(C)))

F32 = mybir.dt.float32
AF = mybir.ActivationFunctionType
ALU = mybir.AluOpType
AX = mybir.AxisListType

_CACHE = {}


def _new_bacc(ncores):
    return bacc.Bacc(
        "TRN2",
        target_bir_lowering=False,
        debug=False,
        enable_asserts=True,
        num_devices=ncores,
    )


def _build_l1(ncores=NCORES):
    nc = _new_bacc(ncores)
    xt_d = nc.dram_tensor("xt", [EP, C, E], F32, kind="ExternalInput").ap()
    out_d = nc.dram_tensor("sqstat", [E, STW], F32, kind="ExternalOutput").ap()

    eyescale_np = (np.eye(E) * -0.5).astype(np.float32)
    eyk_np = (np.eye(E) * (-2.0 * SQC)).astype(np.float32)

    with tile.TileContext(nc) as tc:
        with (
            tc.tile_pool(name="sb", bufs=1) as sb,
            tc.tile_pool(name="scr", bufs=2) as scr,
            tc.tile_pool(name="ps", bufs=1, space="PSUM") as ps,
        ):
            # small consts first (memsets are cheap and dependency-free)
            ones_row = sb.tile([1, E], F32, tag="ones_row")
            nc.vector.memset(ones_row, 1.0)
            shift_col = sb.tile([E, 1], F32, tag="shift_col")
            nc.vector.memset(shift_col, -SHIFT)
            # dependency-free dummy activations pull the ACT table load to t=0
            dummy = sb.tile([1, 1], F32, tag="dummy")
            nc.scalar.activation(dummy, shift_col[0:1, 0:1], AF.Square)

            eyescale = sb.tile([E, E], F32, tag="eyescale")
            nc.gpsimd.dma_start(
                out=eyescale, in_=nc.inline_tensor(eyescale_np, name="c_eyescale").ap()
            )
            eyk = sb.tile([E, E], F32, tag="eyk")
            nc.gpsimd.dma_start(out=eyk, in_=nc.inline_tensor(eyk_np, name="c_eyk").ap())

            # x transposed; ep0 split in two DMAs so its Gram starts earlier
            h = KCHUNKS // 2
            xt0a = sb.tile([E, h, E], F32, tag="xt0a")
            xt0b = sb.tile([E, h, E], F32, tag="xt0b")
            xt_r = xt_d[0].rearrange("(k p) e -> p k e", p=E)
            nc.sync.dma_start(out=xt0a, in_=xt_r[:, 0:h, :])
            nc.sync.dma_start(out=xt0b, in_=xt_r[:, h:KCHUNKS, :])
            xt1 = sb.tile([E, KCHUNKS, E], F32, tag="xt1")
            nc.sync.dma_start(out=xt1, in_=xt_d[1].rearrange("(k p) e -> p k e", p=E))

            def xt_chunk(ep, k):
                if ep == 1:
                    return xt1[:, k, :]
                return (xt0a if k < h else xt0b)[:, k % h, :]

            out_sb = sb.tile([E, STW], F32, tag="out_sb")
            rneg = sb.tile([E, EP], F32, tag="rneg")

            g_ps = []
            for ep in range(EP):
                g = ps.tile([E, E], F32, tag=f"g{ep}")
                for k in range(KCHUNKS):
                    ck = xt_chunk(ep, k)
                    nc.tensor.matmul(g, ck, ck, start=(k == 0), stop=(k == KCHUNKS - 1))
                g_ps.append(g)
                # rneg = -r/2 = rowsum(G * (-eye/2))
                dscratch = scr.tile([E, E], F32, tag="dscratch")
                nc.vector.tensor_mul(dscratch, g, eyescale)
                nc.vector.tensor_reduce(
                    rneg[:, ep : ep + 1], dscratch, axis=AX.X, op=ALU.add
                )

            for ep in range(EP):
                # row of rneg*k via the scaled-eye matmul, broadcast down the
                # partitions with a K=1 outer product against ones
                rr_ps = ps.tile([1, E], F32, tag=f"rr{ep}")
                nc.tensor.matmul(rr_ps, rneg[:, ep : ep + 1], eyk, start=True, stop=True)
                rr = sb.tile([1, E], F32, tag=f"rrow{ep}")
                nc.scalar.copy(rr, rr_ps)
                o_ps = ps.tile([E, E], F32, tag=f"o{ep}")
                nc.tensor.matmul(o_ps, ones_row, rr, start=True, stop=True)

                # sq = (G + rneg_i)*k + o ; diagonal cancels exactly
                t1 = scr.tile([E, E], F32, tag="t1")
                nc.vector.tensor_scalar(
                    t1, g_ps[ep], rneg[:, ep : ep + 1], -2.0 * SQC,
                    op0=ALU.add, op1=ALU.mult,
                )
                sq_slice = out_sb[:, ep * E : (ep + 1) * E]
                nc.vector.tensor_add(sq_slice, t1, o_ps)
                nc.vector.tensor_reduce(
                    out_sb[:, SQW + ep : SQW + ep + 1], sq_slice, axis=AX.X, op=ALU.add
                )
                qscratch = scr.tile([E, E], F32, tag="qscratch")
                nc.scalar.activation(
                    qscratch, sq_slice, AF.Square,
                    bias=shift_col[:, 0:1],
                    accum_out=out_sb[:, SQW + 2 + ep : SQW + 3 + ep],
                )

            nc.sync.dma_start(out=out_d, in_=out_sb)

    nc.compile()
    return nc


def _build_l2(ncores=NCORES):
    nc = _new_bacc(ncores)
    sq_d = nc.dram_tensor("sqn", [E, 1 + SQW], F32, kind="ExternalInput").ap()
    bm_d = nc.dram_tensor("bmat", [EP, E, NB], F32, kind="ExternalInput").ap()
    out_d = nc.dram_tensor("out", [EP, E, NCLASSES], F32, kind="ExternalOutput").ap()

    mask01_np = (1.0 - np.eye(E, dtype=np.float32))
    eye_np = np.eye(E, dtype=np.float32)

    with tile.TileContext(nc) as tc:
        with (
            tc.tile_pool(name="sb", bufs=1) as sb,
            tc.tile_pool(name="ps", bufs=2, space="PSUM") as ps,
        ):
            lnbias_col = sb.tile([E, 1], F32, tag="lnbias_col")
            nc.vector.memset(lnbias_col, EPS_OUT)
            # dependency-free dummies pull the ACT table loads forward; Ln
            # first so the last (resident) set is the one exp needs
            dummy = sb.tile([1, 1], F32, tag="dummy")
            nc.scalar.activation(
                dummy, lnbias_col[0:1, 0:1], AF.Ln, bias=lnbias_col[0:1, 0:1]
            )
            dummy2 = sb.tile([1, 1], F32, tag="dummy2")
            nc.scalar.activation(dummy2, lnbias_col[0:1, 0:1], AF.Exp)

            mask01 = sb.tile([E, E], F32, tag="mask01")
            nc.gpsimd.dma_start(
                out=mask01, in_=nc.inline_tensor(mask01_np, name="c_mask01").ap()
            )
            bm = sb.tile([E, EP, NB], F32, tag="bm")
            nc.gpsimd.dma_start(out=bm, in_=bm_d.rearrange("ep i j -> i ep j"))

            # episode 0's DMA carries nis (host-replicated) in column 0
            sq0n = sb.tile([E, 1 + E], F32, tag="sq0n")
            nc.sync.dma_start(out=sq0n, in_=sq_d[:, 0 : 1 + E])
            nis_col = sq0n[:, 0:1]
            sq1 = sb.tile([E, E], F32, tag="sq1")
            nc.sync.dma_start(out=sq1, in_=sq_d[:, 1 + E : 1 + 2 * E])
            sq = [sq0n[:, 1 : 1 + E], sq1]
            eye = sb.tile([E, E], F32, tag="eye")
            nc.sync.dma_start(out=eye, in_=nc.inline_tensor(eye_np, name="c_eye").ap())

            wz = []
            ts = sb.tile([E, EP], F32, tag="ts")
            for ep in range(EP):
                w = sb.tile([E, E], F32, tag=f"w{ep}")
                nc.scalar.activation(w, sq[ep], AF.Exp, scale=nis_col[:, 0:1])
                wz_t = sb.tile([E, E], F32, tag=f"wz{ep}")
                nc.vector.tensor_mul(wz_t, w, mask01)
                wz.append(wz_t)
                dcol = sb.tile([E, 1], F32, tag=f"dcol{ep}")
                nc.vector.tensor_reduce(dcol, wz_t, axis=AX.X, op=ALU.add)
                dn = sb.tile([E, 1], F32, tag=f"dn{ep}")
                nc.vector.tensor_scalar_add(dn, dcol, EPS_DIAG)
                trec = sb.tile([E, 1], F32, tag=f"trec{ep}")
                nc.vector.reciprocal(trec, dn)
                nc.vector.tensor_scalar_mul(ts[:, ep : ep + 1], trec, ALPHA)

            outv = sb.tile([E, EP, NCLASSES], F32, tag="outv")
            for ep in range(EP):
                bslice = bm[:, ep, :]
                tslice = ts[:, ep : ep + 1]
                u = sb.tile([E, NB], F32, tag=f"u{ep}")
                nc.vector.tensor_scalar_mul(u, bslice, tslice)
                v_ps = None
                for it in range(NEUMANN_ITERS):
                    v_ps = ps.tile([E, NB], F32, tag=f"v{ep}")
                    nc.tensor.matmul(v_ps, eye, bslice, start=True, stop=False)
                    nc.tensor.matmul(v_ps, wz[ep], u, start=False, stop=True)
                    if it < NEUMANN_ITERS - 1:
                        u = sb.tile([E, NB], F32, tag=f"u{ep}")
                        nc.vector.tensor_scalar_mul(u, v_ps, tslice)
                recip_l1 = sb.tile([E, 1], F32, tag=f"rl1{ep}")
                nc.vector.reciprocal(recip_l1, v_ps[:, NCLASSES : NCLASSES + 1])
                # out = Ln(v * (1/l1) + 1e-6) in one ACT op (per-partition scale)
                nc.scalar.activation(
                    outv[:, ep, :], v_ps[:, 0:NCLASSES], AF.Ln,
                    bias=lnbias_col[:, 0:1], scale=recip_l1[:, 0:1],
                )
            nc.sync.dma_start(out=out_d.rearrange("ep i j -> i ep j"), in_=outv)

    nc.compile()
    return nc


def _get(name, builder):
    if name not in _CACHE:
        _CACHE[name] = builder()
    return _CACHE[name]


def _prepare_l1_in_maps(x):
    x = np.ascontiguousarray(np.asarray(x, dtype=np.float32))
    xt = np.ascontiguousarray(x.transpose(0, 2, 1))  # (b, c, e)
    return [
        {"xt": np.ascontiguousarray(xt[c * EP : (c + 1) * EP])} for c in range(NCORES)
    ]


def _host_combine(sqstat_list):
    st = np.stack([s[:, SQW:] for s in sqstat_list]).astype(np.float64)  # (cores,E,4)
    A = float(st[..., 0:EP].sum())
    Q = float(st[..., EP : 2 * EP].sum())
    q_off = Q - B_FULL * E * SHIFT * SHIFT
    d = A - CNT * SHIFT
    var = (q_off - d * d / CNT) / (CNT - 1.0)
    return np.float32(-1.0 / np.sqrt(var))


def _prepare_l2_in_maps(res1, labels, nis):
    labels = np.asarray(labels)
    bmat = np.zeros((B_FULL, E, NB), np.float32)
    bmat[..., NCLASSES] = 1.0
    for j in range(NCLASSES):
        bmat[..., j] = (labels == j).astype(np.float32)
    maps = []
    for c in range(NCORES):
        sqn = np.empty((E, 1 + SQW), np.float32)
        sqn[:, 0] = nis
        sqn[:, 1:] = res1[c]["sqstat"][:, 0:SQW]
        maps.append(
            {
                "sqn": sqn,
                "bmat": np.ascontiguousarray(bmat[c * EP : (c + 1) * EP]),
            }
        )
    return maps


def _run_spmd(nc, in_maps):
    """Run with retries: a crashed predecessor process can leave the
    accelerator in NRT_EXEC_UNIT_UNRECOVERABLE; it recovers on a fresh
    attempt after a short wait."""
    import time

    last = None
    for attempt in range(3):
        try:
            return bass_utils.run_bass_kernel_spmd(
                nc, in_maps, core_ids=list(range(NCORES))
            ).results
        except Exception as e:  # noqa: BLE001 - device transients are opaque
            last = e
            time.sleep(15 * (attempt + 1))
    raise last


def run(inputs):
    nc1 = _get("l1", _build_l1)
    nc2 = _get("l2", _build_l2)
    res1 = _run_spmd(nc1, _prepare_l1_in_maps(inputs["x"]))
    nis = _host_combine([r["sqstat"] for r in res1])
    res2 = _run_spmd(nc2, _prepare_l2_in_maps(res1, inputs["labels"], nis))
    out = np.concatenate([res2[c]["out"] for c in range(NCORES)], axis=0)
    return out.astype(np.float32)


def kernel(x, labels, nclasses):
    assert int(nclasses) == NCLASSES
    return run({"x": x, "labels": labels})


def timeline_estimate(trace_prefix=None):
    """Cost-model (TimelineSim) per-core estimates for both launches."""
    from concourse.timeline_sim import TimelineSim
    from trails.perfetto import LazyPerfetto

    for meth in ("enable_explicit_ordering", "reserve_process_order", "add_counter"):
        if not hasattr(LazyPerfetto, meth):
            setattr(LazyPerfetto, meth, lambda self, *a, **k: None)

    durs = []
    for name, builder in (("l1", _build_l1), ("l2", _build_l2)):
        nc = builder(ncores=1)
        trace = trace_prefix is not None
        tl = TimelineSim(nc, trace=trace)
        dur = tl.simulate()
        if trace and tl.perfetto is not None:
            tl.perfetto.save(f"{trace_prefix}_{name}.pftrace")
        durs.append(dur)
    return durs


if __name__ == "__main__":
    rng = np.random.default_rng(0)
    x = rng.standard_normal((B_FULL, E, C)).astype(np.float32)
    labels = rng.integers(0, NCLASSES + 1, size=(B_FULL, E))
    out = kernel(x, labels, NCLASSES)
    print("out", out.shape, out.dtype, out.min(), out.max())
